# revision 2
# baseline (speedup 1.0000x reference)
"""DGI (Deep Graph Infomax) forward pass on 8 Trainium2 NeuronCores.

Strategy (per spec sharding hint): row-shard the dense adjacency over the
node dimension N across the 8 cores. Each core computes the GCN features
fts = seq @ fc_w.T for all nodes (seq is replicated), then a local GEMM
agg_shard^T = fts^T-stacked @ adjT_shard accumulating both seq1/seq2 paths
in one pass over adj (adj is read once), PReLU, a masked readout partial
sum that is AllReduce'd across cores, and the bilinear discriminator
locally per node shard.

Layout notes:
  - adj is uploaded pre-transposed (adjT [N, NS] per core) and cast to
    bf16 on host so the aggregation contraction index m sits on SBUF
    partitions; bf16 halves the dominant HBM traffic.
  - The 128-row feature axis stacks h1 (rows 0:64) and h2 (rows 64:128)
    so one matmul pass computes both GCN applications.
"""
import sys

if "/opt/trn_rl_repo" not in sys.path:
    sys.path.insert(0, "/opt/trn_rl_repo")

import ml_dtypes
import numpy as np

import concourse.bass as bass  # noqa: F401  (bass types via bacc/tile)
import concourse.mybir as mybir
import concourse.tile as tile
from concourse import bacc, bass_utils

N, F, H, C = 8192, 256, 64, 8
NS = N // C  # 1024 nodes per core
H2 = 2 * H  # stacked h1|h2 feature rows
MT = N // 128  # 64 contraction tiles
TCH = 8  # adj stream chunks
MTC = MT // TCH  # m-tiles per chunk
NCH = NS // 512  # 512-wide output column chunks per core
FO = F // 128  # f-dim tiles

BF16 = mybir.dt.bfloat16
F32 = mybir.dt.float32
NPBF16 = ml_dtypes.bfloat16

_CACHE: dict = {}


def _build():
    nc = bacc.Bacc("TRN2", target_bir_lowering=False, debug=False, num_devices=C)

    adjT_d = nc.dram_tensor("adjT", [N, NS], BF16, kind="ExternalInput").ap()
    sq1T_d = nc.dram_tensor("sq1T", [F, N], BF16, kind="ExternalInput").ap()
    sq2T_d = nc.dram_tensor("sq2T", [F, N], BF16, kind="ExternalInput").ap()
    fcwT_d = nc.dram_tensor("fcwT", [F, H], BF16, kind="ExternalInput").ap()
    bias_d = nc.dram_tensor("bias2", [H2, 1], F32, kind="ExternalInput").ap()
    alpha_d = nc.dram_tensor("alpha2", [H2, 1], F32, kind="ExternalInput").ap()
    invn_d = nc.dram_tensor("invn", [H2, 1], F32, kind="ExternalInput").ap()
    mskb_d = nc.dram_tensor("mskb", [H2, NS], F32, kind="ExternalInput").ap()
    dwTb_d = nc.dram_tensor("dwTb", [H2, H2], F32, kind="ExternalInput").ap()
    blkm_d = nc.dram_tensor("blkm", [H2, 2], F32, kind="ExternalInput").ap()
    db_d = nc.dram_tensor("db2", [2, 1], F32, kind="ExternalInput").ap()
    sc_d = nc.dram_tensor("sc", [2, NS], F32, kind="ExternalOutput").ap()

    with tile.TileContext(nc) as tc:
        with (
            tc.tile_pool(name="const", bufs=1) as constp,
            tc.tile_pool(name="seq", bufs=1) as seqp,
            tc.tile_pool(name="ftsp", bufs=1) as ftsp,
            tc.tile_pool(name="adj", bufs=3) as adjp,
            tc.tile_pool(name="work", bufs=2) as workp,
            tc.tile_pool(name="psf", bufs=2, space="PSUM") as psf,
            tc.tile_pool(name="psh", bufs=1, space="PSUM") as psh,
            tc.tile_pool(name="pss", bufs=1, space="PSUM") as pss,
            tc.tile_pool(name="dram", bufs=1, space="DRAM") as dramp,
        ):
            fcw_sb = constp.tile([128, FO, H], BF16)
            nc.sync.dma_start(fcw_sb[:], fcwT_d.rearrange("(o p) h -> p o h", p=128))
            bias_sb = constp.tile([H2, 1], F32)
            nc.sync.dma_start(bias_sb[:], bias_d[:])
            alpha_sb = constp.tile([H2, 1], F32)
            nc.sync.dma_start(alpha_sb[:], alpha_d[:])
            invn_sb = constp.tile([H2, 1], F32)
            nc.sync.dma_start(invn_sb[:], invn_d[:])
            mskb_sb = constp.tile([H2, NS], F32)
            nc.sync.dma_start(mskb_sb[:], mskb_d[:])
            dwTb_sb = constp.tile([H2, H2], F32)
            nc.sync.dma_start(dwTb_sb[:], dwTb_d[:])
            blkm_sb = constp.tile([H2, 2], F32)
            nc.sync.dma_start(blkm_sb[:], blkm_d[:])
            db_sb = constp.tile([2, 1], F32)
            nc.sync.dma_start(db_sb[:], db_d[:])

            sq1_sb = seqp.tile([128, FO, N], BF16)
            sq2_sb = seqp.tile([128, FO, N], BF16)
            fts_sb = ftsp.tile([128, MT, H2], BF16)
            hs_sb = ftsp.tile([H2, NS], F32)

            ph = [
                psh.tile([H2, 512], F32, tag=f"ph{cn}", name=f"ph{cn}")
                for cn in range(NCH)
            ]

            for t in range(TCH):
                msl = slice(t * (N // TCH), (t + 1) * (N // TCH))
                nc.sync.dma_start(
                    sq1_sb[:, :, msl],
                    sq1T_d[:, msl].rearrange("(o p) m -> p o m", p=128),
                )
                nc.sync.dma_start(
                    sq2_sb[:, :, msl],
                    sq2T_d[:, msl].rearrange("(o p) m -> p o m", p=128),
                )
                adj_sb = adjp.tile([128, MTC, NS], BF16)
                nc.sync.dma_start(
                    adj_sb[:], adjT_d[msl, :].rearrange("(j p) n -> p j n", p=128)
                )
                for j in range(MTC):
                    mt = t * MTC + j
                    mcols = slice(mt * 128, (mt + 1) * 128)
                    pf = psf.tile([128, H2], F32, tag="pf")
                    for fo in range(FO):
                        first, last = fo == 0, fo == FO - 1
                        nc.tensor.matmul(
                            pf[:, 0:H],
                            lhsT=sq1_sb[:, fo, mcols],
                            rhs=fcw_sb[:, fo, :],
                            start=first,
                            stop=last,
                        )
                        nc.tensor.matmul(
                            pf[:, H:H2],
                            lhsT=sq2_sb[:, fo, mcols],
                            rhs=fcw_sb[:, fo, :],
                            start=False,
                            stop=last,
                            skip_group_check=True,
                        )
                    nc.any.tensor_copy(out=fts_sb[:, mt, :], in_=pf[:])
                    for cn in range(NCH):
                        nc.tensor.matmul(
                            ph[cn][:],
                            lhsT=fts_sb[:, mt, :],
                            rhs=adj_sb[:, j, cn * 512 : (cn + 1) * 512],
                            start=(mt == 0),
                            stop=(mt == MT - 1),
                        )

            # PReLU(x + bias), masked readout partials
            s2_sb = workp.tile([H2, NCH], F32, tag="s2")
            for cn in range(NCH):
                nsl = slice(cn * 512, (cn + 1) * 512)
                xb = workp.tile([H2, 512], F32, tag="xb")
                nc.vector.tensor_scalar_add(xb[:], ph[cn][:], bias_sb[:])
                q = workp.tile([H2, 512], F32, tag="q")
                nc.vector.tensor_scalar(
                    q[:],
                    xb[:],
                    0.0,
                    alpha_sb[:],
                    mybir.AluOpType.min,
                    mybir.AluOpType.mult,
                )
                nc.vector.tensor_scalar_max(xb[:], xb[:], 0.0)
                nc.vector.tensor_add(out=hs_sb[:, nsl], in0=xb[:], in1=q[:])
                mskd = workp.tile([H2, 512], F32, tag="mskd")
                nc.vector.tensor_mul(out=mskd[:], in0=hs_sb[:, nsl], in1=mskb_sb[:, nsl])
                nc.vector.tensor_reduce(
                    s2_sb[:, cn : cn + 1],
                    mskd[:],
                    axis=mybir.AxisListType.X,
                    op=mybir.AluOpType.add,
                )
            s_sb = workp.tile([H2, 1], F32, tag="s1")
            nc.vector.tensor_reduce(
                s_sb[:], s2_sb[:], axis=mybir.AxisListType.X, op=mybir.AluOpType.add
            )

            # AllReduce the readout partials across the 8 cores
            s_in = dramp.tile([H2, 1], F32)
            s_out = dramp.tile([H2, 1], F32)
            nc.sync.dma_start(s_in[:], s_sb[:])
            nc.gpsimd.collective_compute(
                "AllReduce",
                mybir.AluOpType.add,
                replica_groups=[list(range(C))],
                ins=[s_in.opt()],
                outs=[s_out.opt()],
            )
            # replicate the h1 readout into both partition halves
            srep_sb = workp.tile([H2, 1], F32, tag="srep")
            nc.sync.dma_start(srep_sb[0:H, :], s_out[0:H, :])
            nc.sync.dma_start(srep_sb[H:H2, :], s_out[0:H, :])

            c_sb = workp.tile([H2, 1], F32, tag="c")
            nc.scalar.activation(
                c_sb[:],
                srep_sb[:],
                mybir.ActivationFunctionType.Sigmoid,
                bias=0.0,
                scale=invn_sb[:],
            )
            pw = pss.tile([H2, 1], F32, tag="pw")
            nc.tensor.matmul(pw[:], lhsT=dwTb_sb[:], rhs=c_sb[:], start=True, stop=True)
            wcb_sb = workp.tile([H2, 2], F32, tag="wcb")
            nc.vector.tensor_mul(
                out=wcb_sb[:], in0=blkm_sb[:], in1=pw[:].to_broadcast([H2, 2])
            )

            out_sb = workp.tile([2, NS], F32, tag="osb")
            for cn in range(NCH):
                nsl = slice(cn * 512, (cn + 1) * 512)
                ps = pss.tile([2, 512], F32, tag="ps")
                nc.tensor.matmul(
                    ps[:], lhsT=wcb_sb[:], rhs=hs_sb[:, nsl], start=True, stop=True
                )
                nc.vector.tensor_scalar_add(out_sb[:, nsl], ps[:], db_sb[:])
            nc.sync.dma_start(sc_d[:], out_sb[:])

    nc.compile()
    return nc


def _get_nc():
    if "nc" not in _CACHE:
        _CACHE["nc"] = _build()
    return _CACHE["nc"]


def kernel(seq1, seq2, adj, msk, fc_w, gcn_bias, prelu_alpha, disc_w, disc_b):
    nc = _get_nc()

    seq1 = np.asarray(seq1, np.float32)
    seq2 = np.asarray(seq2, np.float32)
    adj = np.asarray(adj, np.float32)
    msk = np.asarray(msk, np.float32)
    fc_w = np.asarray(fc_w, np.float32)
    gcn_bias = np.asarray(gcn_bias, np.float32)
    disc_w = np.asarray(disc_w, np.float32)
    disc_b = np.asarray(disc_b, np.float32)

    adj16 = adj[0].astype(NPBF16)  # [N, N]
    sq1T = np.ascontiguousarray(seq1[0].T).astype(NPBF16)
    sq2T = np.ascontiguousarray(seq2[0].T).astype(NPBF16)
    fcwT = np.ascontiguousarray(fc_w.T).astype(NPBF16)
    bias2 = np.concatenate([gcn_bias, gcn_bias]).reshape(H2, 1).astype(np.float32)
    alpha2 = np.full((H2, 1), float(np.asarray(prelu_alpha)), np.float32)
    invn = np.full((H2, 1), 1.0 / float(msk.sum()), np.float32)
    dwTb = np.zeros((H2, H2), np.float32)
    dwTb[0:H, 0:H] = disc_w.T
    dwTb[H:H2, H:H2] = disc_w.T
    blkm = np.zeros((H2, 2), np.float32)
    blkm[0:H, 0] = 1.0
    blkm[H:H2, 1] = 1.0
    db2 = np.full((2, 1), float(disc_b[0]), np.float32)

    in_maps = []
    for i in range(C):
        rows = slice(i * NS, (i + 1) * NS)
        in_maps.append(
            {
                "adjT": np.ascontiguousarray(adj16[rows, :].T),
                "sq1T": sq1T,
                "sq2T": sq2T,
                "fcwT": fcwT,
                "bias2": bias2,
                "alpha2": alpha2,
                "invn": invn,
                "mskb": np.ascontiguousarray(
                    np.broadcast_to(msk[0, rows], (H2, NS))
                ).astype(np.float32),
                "dwTb": dwTb,
                "blkm": blkm,
                "db2": db2,
            }
        )

    res = bass_utils.run_bass_kernel_spmd(nc, in_maps, list(range(C)))
    out = np.empty((1, 2 * N), np.float32)
    for i in range(C):
        sc = res.results[i]["sc"]
        out[0, i * NS : (i + 1) * NS] = sc[0]
        out[0, N + i * NS : N + (i + 1) * NS] = sc[1]
    return out
